# revision 1
# baseline (speedup 1.0000x reference)
"""Trainium2 Bass kernel for nn_ConvolutionLayer (5x5 VALID conv).

Full inputs:  x (16,32,224,224) f32, weight (64,32,5,5) f32, bias (64,) f32
Full output:  (16,64,220,220) f32

Sharding: data-parallel over batch — 2 images per core on 8 cores.

Per-core algorithm (all matmuls in float32r, 1 cyc/row at N>=256):
  - x stored in SBUF as row-quad blocks [128=(r,c), 452=(img,w)+pad] on
    two grids: G0 blocks start at rows 0 mod 4, G1 blocks at rows 2
    mod 4. For any output row h, the 5-row contraction window (rows
    h..h+4, (kh,c) on partitions) splits into one K=128 "main" matmul
    at base 0 plus one K=32 "strip" matmul — 2 matmuls per kw-group.
  - kw packed into M: groups {kw0,kw2} and {kw1,kw3} (M=128: lo half f
    for the even kw, hi half f for the odd kw) — within a group both
    halves share the same rhs column shift, so all 6 matmuls of a row
    accumulate into ONE psum bank P[128,448]:
       P[f,    w] += z0[w]   (+ z1[w+1] via shift-1 group) (+ z4[w+4], M=64)
       P[64+f, w] += z2[w]   (+ z3[w+1])
    giving out[f,h,w] = P[f, w] + P[64+f, w+2] + bias.
    Shifted reads are single-segment across the padded 452-wide block;
    seam bleed lands only in dead columns 220..223 / 444..447.
  - Epilogue: ACT Identity(+bias) moves the lo half to the staging tile,
    VE adds the hi half (PSUM read at base 64) in place. Staging tile
    [128=(img,f), GH*220] is DMA'd to HBM every GH rows.
"""
import sys

sys.path.insert(0, "/opt/trn_rl_repo")

import numpy as np
import concourse.bacc as bacc
import concourse.mybir as mybir
from concourse.tile import TileContext
from concourse.bass_utils import run_bass_kernel_spmd

F32 = mybir.dt.float32
F32R = mybir.dt.float32r

N_CORES = 8
B, C, H, W = 16, 32, 224, 224
F, K = 64, 5
HO, WO = H - K + 1, W - K + 1  # 220, 220
NB = 2                      # images per core
NBLK = H // 4               # 56 row-quad blocks per grid
NW = NB * W                 # 448 data columns per block
NWP = NW + 4                # padded width (even-N fp32r + shifted reads)
GH = 10                     # output rows per staging/DMA group

_cache = {}


def _build(reps=1, xbufs=4, psbufs=4, stbufs=2):
    nc = bacc.Bacc(trn_type="TRN2")

    xg0 = nc.dram_tensor("xg0", [NBLK, 128, NWP], F32R, kind="ExternalInput")
    xg1 = nc.dram_tensor("xg1", [NBLK, 128, NWP], F32R, kind="ExternalInput")
    w02 = nc.dram_tensor("w02", [160, 128], F32R, kind="ExternalInput")
    w13 = nc.dram_tensor("w13", [160, 128], F32R, kind="ExternalInput")
    w4 = nc.dram_tensor("w4", [160, 64], F32R, kind="ExternalInput")
    bias = nc.dram_tensor("bias", [64, 1], F32, kind="ExternalInput")
    out = nc.dram_tensor("out", [NB, F, HO, WO], F32, kind="ExternalOutput")

    with TileContext(nc) as tc:
        with (
            tc.tile_pool(name="wp", bufs=1) as wp,
            tc.tile_pool(name="bp", bufs=1) as bp,
            tc.tile_pool(name="xp", bufs=xbufs) as xp,
            tc.tile_pool(name="pp", bufs=psbufs, space="PSUM") as pp,
            tc.tile_pool(name="op", bufs=stbufs) as op,
        ):
            # ---- weights ----
            # per kw-group g: T1 = Wmat[0:128]@0 (kh0..3), T2 = Wmat[32:160]@0
            # (kh1..4); t345: Wmat[128:160]@0 (kh4), Wmat[0:32]@32, @96 (kh0).
            wt = {}
            for name, wd, m in (("02", w02, 128), ("13", w13, 128), ("4", w4, 64)):
                t1 = wp.tile([128, m], F32R, tag=f"t1{name}")
                t2 = wp.tile([128, m], F32R, tag=f"t2{name}")
                t345 = wp.tile([128, m], F32R, tag=f"t345{name}")
                nc.sync.dma_start(out=t1[:], in_=wd[0:128, :])
                nc.sync.dma_start(out=t2[:], in_=wd[32:160, :])
                nc.sync.dma_start(out=t345[0:32, :], in_=wd[128:160, :])
                nc.sync.dma_start(out=t345[32:64, :], in_=wd[0:32, :])
                nc.sync.dma_start(out=t345[96:128, :], in_=wd[0:32, :])
                wt[name] = (t1, t2, t345)
            bt = bp.tile([64, 1], F32)
            nc.sync.dma_start(out=bt[:], in_=bias[:])

            g0_tiles, g1_tiles = {}, {}

            def load_block(store, src, b):
                t = xp.tile([128, NWP], F32R, tag=f"x{'0' if src is xg0 else '1'}")
                nc.sync.dma_start(out=t[:], in_=src[b, :, :])
                store[b] = t

            groups = (("02", 128, 0), ("13", 128, 1), ("4", 64, 4))

            def emit_pass():
                g0_tiles.clear()
                g1_tiles.clear()
                load_block(g0_tiles, xg0, 0)
                load_block(g1_tiles, xg1, 0)
                stage = None
                for b in range(55):
                    load_block(g0_tiles, xg0, b + 1)
                    load_block(g1_tiles, xg1, b + 1)
                    for r in range(4):
                        h = 4 * b + r
                        if h % GH == 0:
                            stage = op.tile([128, GH * WO], F32, tag="stage")
                        col = (h % GH) * WO

                        if r == 0:
                            main, wmain = g0_tiles[b], 0        # kh0..3 -> T1
                            strip, sbase = g0_tiles[b + 1], 0   # kh4 -> T3
                        elif r == 1:
                            main, wmain = g1_tiles[b], 1        # kh1..4 -> T2
                            strip, sbase = g0_tiles[b], 32      # kh0 -> T4
                        elif r == 2:
                            main, wmain = g1_tiles[b], 0
                            strip, sbase = g1_tiles[b + 1], 0
                        else:
                            main, wmain = g0_tiles[b + 1], 1
                            strip, sbase = g0_tiles[b], 96      # kh0 -> T5
                        ps = pp.tile([128, NW], F32, tag="ps")

                        first = True
                        for gi, (gname, m, sh) in enumerate(groups):
                            t1, t2, t345 = wt[gname]
                            wm = t1 if wmain == 0 else t2
                            rhs_m = main[0:128, sh:sh + NW]
                            rhs_s = strip[sbase:sbase + 32, sh:sh + NW]
                            out_ap = ps[0:m, 0:NW]
                            last = gi == len(groups) - 1
                            nc.tensor.matmul(out=out_ap, lhsT=wm[0:128, 0:m],
                                             rhs=rhs_m, start=first, stop=False)
                            nc.tensor.matmul(
                                out=out_ap, lhsT=t345[sbase:sbase + 32, 0:m],
                                rhs=rhs_s, start=False, stop=last,
                                tile_position=(sbase, 0))
                            first = False

                        # out[f,h,w] = ps[f, n*224+w] + ps[64+f, n*224+w+2] + b
                        for n in range(NB):
                            o_lo = stage[64 * n:64 * n + 64, col:col + WO]
                            nc.scalar.activation(
                                out=o_lo, in_=ps[0:64, n * W:n * W + WO],
                                func=mybir.ActivationFunctionType.Identity,
                                bias=bt[:], scale=1.0)
                            nc.vector.tensor_add(
                                out=o_lo,
                                in0=ps[64:128, n * W + 2:n * W + 2 + WO],
                                in1=o_lo)

                        if h % GH == GH - 1:
                            h0 = h - GH + 1
                            nc.sync.dma_start(
                                out=out[:, :, h0:h0 + GH, :].rearrange(
                                    "n f h w -> (n f) (h w)"),
                                in_=stage[:],
                            )

            for _ in range(reps):
                emit_pass()

    nc.finalize()
    return nc


def _prep_core(xs, weight, bias):
    """xs: (2,32,224,224) -> per-core input map."""
    def _grid(arr):
        g = arr.reshape(NB, C, NBLK, 4, W).transpose(2, 3, 1, 0, 4)
        o = np.zeros((NBLK, 128, NWP), np.float32)
        o[:, :, :NW] = g.reshape(NBLK, 128, NW)
        return o

    g0 = _grid(xs)
    xpad = np.concatenate(
        [xs[:, :, 2:, :], np.zeros((NB, C, 2, W), np.float32)], axis=2)
    g1 = _grid(xpad)
    # Wmat[32*kh + c, j*64 + f] = weight[f, c, kh, kw_j]
    wm = weight.transpose(2, 1, 0, 3).reshape(160, 64, 5)
    w02 = np.concatenate([wm[:, :, 0], wm[:, :, 2]], axis=1)
    w13 = np.concatenate([wm[:, :, 1], wm[:, :, 3]], axis=1)
    w4 = np.ascontiguousarray(wm[:, :, 4])
    return {
        "xg0": g0,
        "xg1": g1,
        "w02": np.ascontiguousarray(w02),
        "w13": np.ascontiguousarray(w13),
        "w4": w4,
        "bias": bias.reshape(64, 1).astype(np.float32),
    }


def kernel(x, weight, bias, _profile=False):
    x = np.asarray(x, dtype=np.float32)
    weight = np.asarray(weight, dtype=np.float32)
    bias = np.asarray(bias, dtype=np.float32)

    if "nc" not in _cache:
        _cache["nc"] = _build()
    nc = _cache["nc"]

    in_maps = [
        _prep_core(x[NB * i:NB * i + NB], weight, bias) for i in range(N_CORES)
    ]
    res = run_bass_kernel_spmd(
        nc, in_maps, core_ids=list(range(N_CORES)), trace=_profile)
    out = np.concatenate([r["out"] for r in res.results], axis=0)
    if _profile:
        _cache["last_results"] = res
    return out


if __name__ == "__main__":
    rng = np.random.default_rng(0)
    x = rng.standard_normal((B, C, H, W), dtype=np.float32)
    w = rng.standard_normal((F, C, K, K), dtype=np.float32)
    bv = rng.standard_normal((F,), dtype=np.float32)
    o = kernel(x, w, bv)
    print("output shape:", o.shape, o.dtype)



# revision 3
# speedup vs baseline: 5.2505x; 5.2505x over previous
"""Trainium2 Bass kernel for nn_ConvolutionLayer (5x5 VALID conv).

Full inputs:  x (16,32,224,224) f32, weight (64,32,5,5) f32, bias (64,) f32
Full output:  (16,64,220,220) f32

Sharding: data-parallel over batch — 2 images per core on 8 cores.

Per-core algorithm (all matmuls in float32r, 1 cyc/row at N>=256):
  - x stored in SBUF as row-quad blocks [128=(r,c), 452=(img,w)+pad] on
    two grids: G0 blocks start at rows 0 mod 4, G1 blocks at rows 2
    mod 4. For any output row h, the 5-row contraction window (rows
    h..h+4, (kh,c) on partitions) splits into one K=128 "main" matmul
    at base 0 plus one K=32 "strip" matmul — 2 matmuls per kw-group.
  - kw packed into M: groups {kw0,kw2} and {kw1,kw3} (M=128: lo half f
    for the even kw, hi half f for the odd kw) — within a group both
    halves share the same rhs column shift, so all 6 matmuls of a row
    accumulate into ONE psum bank P[128,448]:
       P[f,    w] += z0[w]   (+ z1[w+1] via shift-1 group) (+ z4[w+4], M=64)
       P[64+f, w] += z2[w]   (+ z3[w+1])
    giving out[f,h,w] = P[f, w] + P[64+f, w+2] + bias.
    Shifted reads are single-segment across the padded 452-wide block;
    seam bleed lands only in dead columns 220..223 / 444..447.
  - Epilogue: ACT Identity(+bias) moves the lo half to the staging tile,
    VE adds the hi half (PSUM read at base 64) in place. Staging tile
    [128=(img,f), GH*220] is DMA'd to HBM every GH rows.
"""
import sys

sys.path.insert(0, "/opt/trn_rl_repo")

import numpy as np
import ml_dtypes
import concourse.bacc as bacc
import concourse.mybir as mybir
from concourse.tile import TileContext
from concourse.bass_utils import run_bass_kernel_spmd

F32 = mybir.dt.float32
MMDT = mybir.dt.bfloat16   # matmul operand dtype (PSUM accum stays f32)
NPDT = ml_dtypes.bfloat16

N_CORES = 8
B, C, H, W = 16, 32, 224, 224
F, K = 64, 5
HO, WO = H - K + 1, W - K + 1  # 220, 220
NB = 2                      # images per core
NBLK = H // 4               # 56 row-quad blocks per grid
NW = NB * W                 # 448 data columns per block
NWP = NW + 4                # padded width (even-N fp32r + shifted reads)
GH = 10                     # output rows per staging/DMA group

_cache = {}


def _build(reps=1, xbufs=4, psbufs=4, stbufs=2):
    nc = bacc.Bacc(trn_type="TRN2")

    xg0 = nc.dram_tensor("xg0", [NBLK, 128, NWP], MMDT, kind="ExternalInput")
    xg1 = nc.dram_tensor("xg1", [NBLK, 128, NWP], MMDT, kind="ExternalInput")
    w02 = nc.dram_tensor("w02", [160, 128], MMDT, kind="ExternalInput")
    w13 = nc.dram_tensor("w13", [160, 128], MMDT, kind="ExternalInput")
    w4 = nc.dram_tensor("w4", [160, 64], MMDT, kind="ExternalInput")
    bias = nc.dram_tensor("bias", [64, 1], F32, kind="ExternalInput")
    out = nc.dram_tensor("out", [NB, F, HO, WO], F32, kind="ExternalOutput")

    with TileContext(nc) as tc:
        with (
            tc.tile_pool(name="wp", bufs=1) as wp,
            tc.tile_pool(name="bp", bufs=1) as bp,
            tc.tile_pool(name="xp", bufs=xbufs) as xp,
            tc.tile_pool(name="pp", bufs=psbufs, space="PSUM") as pp,
            tc.tile_pool(name="op", bufs=stbufs) as op,
        ):
            # ---- weights ----
            # per kw-group g: T1 = Wmat[0:128]@0 (kh0..3), T2 = Wmat[32:160]@0
            # (kh1..4); t345: Wmat[128:160]@0 (kh4), Wmat[0:32]@32, @96 (kh0).
            wt = {}
            for name, wd, m in (("02", w02, 128), ("13", w13, 128), ("4", w4, 64)):
                t1 = wp.tile([128, m], MMDT, tag=f"t1{name}")
                t2 = wp.tile([128, m], MMDT, tag=f"t2{name}")
                t345 = wp.tile([128, m], MMDT, tag=f"t345{name}")
                nc.sync.dma_start(out=t1[:], in_=wd[0:128, :])
                nc.sync.dma_start(out=t2[:], in_=wd[32:160, :])
                nc.sync.dma_start(out=t345[0:32, :], in_=wd[128:160, :])
                nc.sync.dma_start(out=t345[32:64, :], in_=wd[0:32, :])
                nc.sync.dma_start(out=t345[96:128, :], in_=wd[0:32, :])
                wt[name] = (t1, t2, t345)
            bt = bp.tile([64, 1], F32)
            nc.sync.dma_start(out=bt[:], in_=bias[:])

            g0_tiles, g1_tiles = {}, {}

            def load_block(store, src, b):
                t = xp.tile([128, NWP], MMDT, tag=f"x{'0' if src is xg0 else '1'}")
                nc.sync.dma_start(out=t[:], in_=src[b, :, :])
                store[b] = t

            groups = (("02", 128, 0), ("13", 128, 1), ("4", 64, 4))

            def emit_pass():
                g0_tiles.clear()
                g1_tiles.clear()
                load_block(g0_tiles, xg0, 0)
                load_block(g1_tiles, xg1, 0)
                stage = None
                for b in range(55):
                    load_block(g0_tiles, xg0, b + 1)
                    load_block(g1_tiles, xg1, b + 1)
                    for r in range(4):
                        h = 4 * b + r
                        if h % GH == 0:
                            stage = op.tile([128, GH * WO], F32, tag="stage")
                        col = (h % GH) * WO

                        if r == 0:
                            main, wmain = g0_tiles[b], 0        # kh0..3 -> T1
                            strip, sbase = g0_tiles[b + 1], 0   # kh4 -> T3
                        elif r == 1:
                            main, wmain = g1_tiles[b], 1        # kh1..4 -> T2
                            strip, sbase = g0_tiles[b], 32      # kh0 -> T4
                        elif r == 2:
                            main, wmain = g1_tiles[b], 0
                            strip, sbase = g1_tiles[b + 1], 0
                        else:
                            main, wmain = g0_tiles[b + 1], 1
                            strip, sbase = g0_tiles[b], 96      # kh0 -> T5
                        ps = pp.tile([128, NW], F32, tag="ps")

                        first = True
                        for gi, (gname, m, sh) in enumerate(groups):
                            t1, t2, t345 = wt[gname]
                            wm = t1 if wmain == 0 else t2
                            rhs_m = main[0:128, sh:sh + NW]
                            rhs_s = strip[sbase:sbase + 32, sh:sh + NW]
                            out_ap = ps[0:m, 0:NW]
                            last = gi == len(groups) - 1
                            nc.tensor.matmul(out=out_ap, lhsT=wm[0:128, 0:m],
                                             rhs=rhs_m, start=first, stop=False)
                            nc.tensor.matmul(
                                out=out_ap, lhsT=t345[sbase:sbase + 32, 0:m],
                                rhs=rhs_s, start=False, stop=last,
                                tile_position=(sbase, 0))
                            first = False

                        # out[f,h,w] = ps[f, n*224+w] + ps[64+f, n*224+w+2] + b
                        for n in range(NB):
                            o_lo = stage[64 * n:64 * n + 64, col:col + WO]
                            nc.scalar.activation(
                                out=o_lo, in_=ps[0:64, n * W:n * W + WO],
                                func=mybir.ActivationFunctionType.Identity,
                                bias=bt[:], scale=1.0)
                            nc.vector.tensor_add(
                                out=o_lo,
                                in0=ps[64:128, n * W + 2:n * W + 2 + WO],
                                in1=o_lo)

                        if h % GH == GH - 1:
                            h0 = h - GH + 1
                            nc.sync.dma_start(
                                out=out[:, :, h0:h0 + GH, :].rearrange(
                                    "n f h w -> (n f) (h w)"),
                                in_=stage[:],
                            )

            for _ in range(reps):
                emit_pass()

    nc.finalize()
    return nc


def _prep_core(xs, weight, bias):
    """xs: (2,32,224,224) -> per-core input map."""
    def _grid(arr):
        g = arr.astype(NPDT).reshape(NB, C, NBLK, 4, W).transpose(2, 3, 1, 0, 4)
        o = np.zeros((NBLK, 128, NWP), NPDT)
        o[:, :, :NW] = g.reshape(NBLK, 128, NW)
        return o

    g0 = _grid(xs)
    xpad = np.concatenate(
        [xs[:, :, 2:, :], np.zeros((NB, C, 2, W), np.float32)], axis=2)
    g1 = _grid(xpad)
    # Wmat[32*kh + c, j*64 + f] = weight[f, c, kh, kw_j]
    wm = weight.astype(NPDT).transpose(2, 1, 0, 3).reshape(160, 64, 5)
    w02 = np.concatenate([wm[:, :, 0], wm[:, :, 2]], axis=1)
    w13 = np.concatenate([wm[:, :, 1], wm[:, :, 3]], axis=1)
    w4 = np.ascontiguousarray(wm[:, :, 4])
    return {
        "xg0": g0,
        "xg1": g1,
        "w02": np.ascontiguousarray(w02),
        "w13": np.ascontiguousarray(w13),
        "w4": w4,
        "bias": bias.reshape(64, 1).astype(np.float32),
    }


def kernel(x, weight, bias, _profile=False):
    x = np.asarray(x, dtype=np.float32)
    weight = np.asarray(weight, dtype=np.float32)
    bias = np.asarray(bias, dtype=np.float32)

    if "nc" not in _cache:
        _cache["nc"] = _build()
    nc = _cache["nc"]

    in_maps = [
        _prep_core(x[NB * i:NB * i + NB], weight, bias) for i in range(N_CORES)
    ]
    res = run_bass_kernel_spmd(
        nc, in_maps, core_ids=list(range(N_CORES)), trace=_profile)
    out = np.concatenate([r["out"] for r in res.results], axis=0)
    if _profile:
        _cache["last_results"] = res
    return out


if __name__ == "__main__":
    rng = np.random.default_rng(0)
    x = rng.standard_normal((B, C, H, W), dtype=np.float32)
    w = rng.standard_normal((F, C, K, K), dtype=np.float32)
    bv = rng.standard_normal((F,), dtype=np.float32)
    o = kernel(x, w, bv)
    print("output shape:", o.shape, o.dtype)

